# revision 38
# baseline (speedup 1.0000x reference)
"""Trainium2 Bass kernel for GQA attention (nn_Attention_75539884802796).

Sharding: data-parallel over batch — B=8 batch elements across 8 NeuronCores,
one full attention layer per core, zero collectives.

Per-core pipeline (static/unrolled, Tile-scheduled, bf16 matmul path):
  1. k/q/v projections as bf16 matmuls (xT resident, weights streamed once,
     per-k-tile DMAs so the PE starts as soon as the first tiles land)
  2. RoPE in natural [token, head*dim] layout on DVE (PSUM evacuated with one
     copy so projection banks free early); k/v caches stored in f32
  3. PE transposes q,k into per-head base-0 [64, S] layouts
  4. per head (software-pipelined, finalize lags 2 heads):
     probsT = exp(kT.T @ qT) computed chunk-wise directly in transposed
     layout (no probs transposes), causal via suffix widths + one diag
     0/1-mask multiply; AV matmul uses V with an appended ones-column so
     row 64 of the [65, 512] PSUM output accumulates softmax denominators;
     1/sums via ACT ln -> exp(-x) (DVE reciprocal is 8 cyc/elem — too slow),
     broadcast across partitions with a f32r ones-outer-product on PE,
     final normalize on DVE writes the transposed attention output
  5. o_proj with a split contraction: the first half (heads 0-15) runs
     during the last 8 attention heads with partials parked in internal
     DRAM; the second half adds them back, shrinking the serial tail
Outputs: out [S, HIDDEN] f32, k/v caches [HKV, S, D] f32 per batch.
Measured: ~282 us exec on hardware, rel err ~5e-3 (bf16 path).

Self-contained: hardcodes all shapes; no sibling imports.

Toolchain compatibility patches (this image's walrus):
  - sync waits are limited to 1 per instruction (0 for Drain/NoOp); Tile
    fuses many waits onto one instruction -> split them into standalone
    EventSemaphore instructions at BIR-JSON serialization time.
  - Tile's tail drain+barrier emits Drains carrying sync -> replaced with
    single-wait instructions and sem-only barriers.
  - gpsimd cast-DMAs and custom gpsimd ISA ops (partition_broadcast) crash
    or fail codegen here — avoided by design.
"""

import ml_dtypes
import numpy as np
import orjson

import concourse.bass as bass
import concourse.mybir as mybir
from concourse.bass_utils import run_bass_kernel_spmd
from concourse.tile import TileContext
from concourse.vector_clock import ScopedClock

HIDDEN = 2048
N_HEADS = 32
N_KV_HEADS = 8
HEAD_DIM = 64
S = 512
B = 8
ROPE_BASE = 10000.0

KT = HIDDEN // 128        # 16 contraction tiles
TT = S // 128             # 4 token tiles
QCH = N_HEADS * HEAD_DIM // 512   # 4 q-projection column chunks (8 heads each)
GROUPS = N_HEADS // N_KV_HEADS

F32 = mybir.dt.float32
BF16 = mybir.dt.bfloat16
AX = mybir.AluOpType
BF = ml_dtypes.bfloat16


# --------------------------------------------------------------------------
# toolchain compatibility patches
# --------------------------------------------------------------------------

def _patch_tile_tail():
    if getattr(TileContext, "_tail_patched", False):
        return

    def patched(self, tick_clock, wait_clock):
        nc = self.nc
        tmp = nc.sync.nop(nofuse=True)
        wait_clock.add_sem_waits(tmp.ins, ScopedClock({None: tick_clock.global_clock}))
        waits = list(tmp.ins.sync_info.on_wait)
        del tmp.ins.sync_info.on_wait[:]
        id2sem = {sem.num: sem for sem in self.sems.allocated().values()}
        for w in waits:
            sem = id2sem.get(w.id)
            assert sem is not None, f"unknown sem id {w.id}"
            nc.sync.wait_ge(sem, w.wait_value)
        nc.all_engine_barrier(sem_only=True)
        popped = nc._tile_sem_poison_stack.pop()
        assert popped is self._sem_poison
        nc.clear_and_free_semaphores(list(self.sems.allocated().values()))
        nc.all_engine_barrier(sem_only=True)

    TileContext._drain_and_barrier = patched
    TileContext._tail_patched = True


def _split_multi_waits(bir: dict) -> dict:
    """Walrus here accepts at most one sync wait per instruction (none on
    Drain/NoOp). Hoist extra waits onto standalone EventSemaphore
    instructions inserted just before, on the same engine."""
    n_new = 0
    for fn in bir.get("functions", []):
        for blk in fn.get("blocks", []):
            insts = blk.get("instructions")
            if not insts:
                continue
            out = []
            for inst in insts:
                si = inst.get("sync_info")
                waits = (si or {}).get("on_wait") or []
                keep = 0 if inst.get("opcode") in ("Drain", "NoOp") else 1
                if len(waits) > keep:
                    split = waits[: len(waits) - keep]
                    si["on_wait"] = waits[len(waits) - keep:]
                    for w in split:
                        n_new += 1
                        out.append({
                            "debug": inst.get("debug", {}),
                            "engine": inst["engine"],
                            "ins": [],
                            "name": f"{inst['name']}_sw{n_new}",
                            "opcode": "EventSemaphore",
                            "outs": [],
                            "sync_info": {"on_update": [], "on_wait": [w]},
                        })
                out.append(inst)
            blk["instructions"] = out
    return bir


def _patch_to_json():
    if getattr(bass.Bass, "_json_multiwait_patched", False):
        return
    orig = bass.Bass.to_json_bytes

    def patched(self):
        data = orig(self)
        bir = orjson.loads(data)
        bir = _split_multi_waits(bir)
        return orjson.dumps(bir)

    bass.Bass.to_json_bytes = patched
    bass.Bass._json_multiwait_patched = True


def apply_patches():
    _patch_tile_tail()
    _patch_to_json()


# --------------------------------------------------------------------------
# kernel graph
# --------------------------------------------------------------------------

def build_nc():
    """Build the per-core Bass graph (same graph on all 8 cores)."""
    apply_patches()
    nc = bass.Bass("TRN2", target_bir_lowering=False)

    xt_d = nc.dram_tensor("xt", [HIDDEN, S], BF16, kind="ExternalInput")
    wq_d = nc.dram_tensor("wq", [HIDDEN, N_HEADS * HEAD_DIM], BF16, kind="ExternalInput")
    wk_d = nc.dram_tensor("wk", [HIDDEN, N_KV_HEADS * HEAD_DIM], BF16, kind="ExternalInput")
    wv_d = nc.dram_tensor("wv", [HIDDEN, N_KV_HEADS * HEAD_DIM], BF16, kind="ExternalInput")
    wo_d = nc.dram_tensor("wo", [N_HEADS * HEAD_DIM, HIDDEN], BF16, kind="ExternalInput")
    cos_d = nc.dram_tensor("cos8", [S, 512], BF16, kind="ExternalInput")
    sin_d = nc.dram_tensor("sin8", [S, 512], BF16, kind="ExternalInput")
    id_d = nc.dram_tensor("ident", [128, 128], BF16, kind="ExternalInput")
    dm_d = nc.dram_tensor("dmask", [128, 128], BF16, kind="ExternalInput")
    on_d = nc.dram_tensor("ones64", [1, 64], mybir.dt.float32r, kind="ExternalInput")

    out_d = nc.dram_tensor("out", [S, HIDDEN], F32, kind="ExternalOutput")
    oacc_d = nc.dram_tensor("oacc", [S, HIDDEN], F32)
    kc_d = nc.dram_tensor("kc", [N_KV_HEADS, S, HEAD_DIM], F32, kind="ExternalOutput")
    vc_d = nc.dram_tensor("vc", [N_KV_HEADS, S, HEAD_DIM], F32, kind="ExternalOutput")

    F32R = mybir.dt.float32r

    with TileContext(nc) as tc:
        with (
            tc.tile_pool(name="const", bufs=1) as cpool,
            tc.tile_pool(name="resident", bufs=1) as rpool,
            tc.tile_pool(name="wstream", bufs=4) as wpool,
            tc.tile_pool(name="work", bufs=3) as work,
            tc.tile_pool(name="probsT_sb", bufs=8) as pT_pool,
            tc.tile_pool(name="stats", bufs=2) as stat,
            tc.tile_pool(name="psProj", bufs=1, space="PSUM") as psP,
            tc.tile_pool(name="psR", bufs=2, space="PSUM") as psR,
            tc.tile_pool(name="psO", bufs=2, space="PSUM") as psO,
        ):
            # ---- constants / resident tensors ----
            ident = cpool.tile([128, 128], BF16, name="ident")
            nc.sync.dma_start(ident[:], id_d[:])
            dmask = cpool.tile([128, 128], BF16, name="dmask")
            nc.sync.dma_start(dmask[:], dm_d[:])
            ones64 = cpool.tile([1, 64], F32R, name="ones64")
            nc.sync.dma_start(ones64[:], on_d[:])
            xt = rpool.tile([128, KT, S], BF16, name="xt")
            xt_r = xt_d.rearrange("(t p) m -> p t m", p=128)
            wq_r = wq_d.rearrange("(t p) n -> p t n", p=128)
            wk_t0 = wpool.tile([128, KT, 512], BF16, name="wk", tag="wtile")
            wk_r0 = wk_d.rearrange("(t p) n -> p t n", p=128)
            wq_t00 = wpool.tile([128, KT, 512], BF16, name="wq_0", tag="wtile")
            for kt in range(KT):
                nc.sync.dma_start(xt[:, kt, :], xt_r[:, kt, :])
                nc.sync.dma_start(wk_t0[:, kt, :], wk_r0[:, kt, :])
            for q4 in range(4):
                nc.sync.dma_start(wq_t00[:, 4 * q4:4 * q4 + 4, :],
                                  wq_r[:, 4 * q4:4 * q4 + 4, 0:512])
            cos8 = cpool.tile([128, TT, 512], BF16, name="cos8")
            nc.sync.dma_start(cos8[:], cos_d.rearrange("(t p) n -> p t n", p=128))
            sin8 = cpool.tile([128, TT, 512], BF16, name="sin8")
            nc.sync.dma_start(sin8[:], sin_d.rearrange("(t p) n -> p t n", p=128))
            wv_t0 = wpool.tile([128, KT, 512], BF16, name="wv", tag="wtile")
            wv_r0 = wv_d.rearrange("(t p) n -> p t n", p=128)
            for q4 in range(4):
                nc.sync.dma_start(wv_t0[:, 4 * q4:4 * q4 + 4, :],
                                  wv_r0[:, 4 * q4:4 * q4 + 4, :])

            qT = rpool.tile([64, N_HEADS, S], BF16, name="qT")
            kT = rpool.tile([64, N_KV_HEADS, S], BF16, name="kT")
            v_sb = rpool.tile([128, TT, N_KV_HEADS, 65], BF16, name="v_sb")
            aoT = rpool.tile([128, KT, S], BF16, name="aoT")
            nc.vector.memset(v_sb[:, :, :, 64:65], 1.0)

            def rope_block(ps_in, cos_t, sin_t, out_tile):
                raw = work.tile([128, 512], F32, name="roperaw", tag="roperaw")
                nc.vector.tensor_copy(raw[:], ps_in)
                qc = work.tile([128, 512], F32, name="ropeqc", tag="ropeqc")
                nc.vector.tensor_tensor(qc[:], raw[:], cos_t, op=AX.mult)
                ps3 = raw.rearrange("p (g c) -> p g c", c=64)
                qc3 = qc.rearrange("p (g c) -> p g c", c=64)
                sn3 = sin_t.rearrange("p (g c) -> p g c", c=64)
                ot3 = out_tile.rearrange("p (g c) -> p g c", c=64)
                t1 = work.tile([128, 256], F32, name="ropet1", tag="ropet1")
                t13 = t1.rearrange("p (g c) -> p g c", c=32)
                nc.vector.tensor_tensor(t13[:], ps3[:, :, 32:64], sn3[:, :, 0:32], op=AX.mult)
                nc.vector.tensor_tensor(ot3[:, :, 0:32], qc3[:, :, 0:32], t13[:], op=AX.subtract)
                t2 = work.tile([128, 256], F32, name="ropet2", tag="ropet2")
                t23 = t2.rearrange("p (g c) -> p g c", c=32)
                nc.vector.tensor_tensor(t23[:], ps3[:, :, 0:32], sn3[:, :, 32:64], op=AX.mult)
                nc.vector.tensor_tensor(ot3[:, :, 32:64], qc3[:, :, 32:64], t23[:], op=AX.add)

            def proj_halves(w_tile, dest_cb):
                """Run a [*,512]-wide projection in two tt-halves on 2 PSUM
                banks; dest_cb(t, ps) consumes each token-tile's PSUM."""
                for half in range(2):
                    ps2 = [psP.tile([128, 512], F32, name=f"pp_{nc.next_id()}",
                                    tag=f"psq{t}") for t in range(2)]
                    for kt in range(KT):
                        for ti in range(2):
                            t = 2 * half + ti
                            nc.tensor.matmul(
                                ps2[ti][:], xt[:, kt, t * 128:(t + 1) * 128],
                                w_tile[:, kt, :],
                                start=(kt == 0), stop=(kt == KT - 1),
                            )
                    for ti in range(2):
                        dest_cb(2 * half + ti, ps2[ti])

            # ================= k projection =================
            wk_t = wk_t0
            kc_r = kc_d.rearrange("g s d -> s g d")

            def k_dest(t, ps_k):
                krot_f = work.tile([128, 512], F32, name="krot_f", tag="rot_f")
                rope_block(ps_k[:], cos8[:, t, :], sin8[:, t, :], krot_f)
                nc.sync.dma_start(
                    kc_r[t * 128:(t + 1) * 128, :, :],
                    krot_f.rearrange("p (g d) -> p g d", d=64),
                )
                krot = work.tile([128, 512], BF16, name="krot", tag="rot")
                nc.vector.tensor_copy(krot[:], krot_f[:])
                ps_trk = psO.tile([64, 8, 128], BF16, name=f"ps_trk_{t}",
                                  tag="rb_ps", bufs=1)
                for gg in range(8):
                    nc.tensor.transpose(ps_trk[:, gg, :],
                                        krot[:, gg * 64:(gg + 1) * 64], ident[:])
                nc.vector.tensor_copy(kT[:, :, t * 128:(t + 1) * 128], ps_trk[:])

            proj_halves(wk_t, k_dest)

            # ================= q chunk 0 projection =================
            def q_dest_factory(o):
                def q_dest(t, ps_q):
                    qrot = work.tile([128, 512], BF16, name="qrot", tag="rot")
                    rope_block(ps_q[:], cos8[:, t, :], sin8[:, t, :], qrot)
                    ps_tr = psO.tile([64, 8, 128], BF16,
                                      name=f"ps_trq_{o}_{t}", tag="rb_ps", bufs=1)
                    for hh in range(8):
                        nc.tensor.transpose(ps_tr[:, hh, :],
                                            qrot[:, hh * 64:(hh + 1) * 64], ident[:])
                    nc.vector.tensor_copy(
                        qT[:, 8 * o:8 * o + 8, t * 128:(t + 1) * 128], ps_tr[:]
                    )
                return q_dest

            proj_halves(wq_t00, q_dest_factory(0))

            # ================= v projection =================
            wv_t = wv_t0
            vc_r = vc_d.rearrange("g s d -> s g d")

            def v_dest(t, ps_v):
                v_f = work.tile([128, 512], F32, name="v_f", tag="rot_f")
                nc.vector.tensor_copy(v_f[:], ps_v[:])
                nc.sync.dma_start(
                    vc_r[t * 128:(t + 1) * 128, :, :],
                    v_f.rearrange("p (g d) -> p g d", d=64),
                )
                nc.vector.tensor_copy(
                    v_sb[:, t, :, 0:64], v_f.rearrange("p (g d) -> p g d", d=64),
                )

            proj_halves(wv_t, v_dest)

            # ======== remaining q chunks interleaved with attention ========
            def attention_head_main(h):
                g = h // GROUPS
                pT_sb = []
                for j in range(TT):
                    wdt = 512 - j * 128
                    ps_r = psR.tile([128, 512], F32, name=f"ps_r_{h}_{j}", tag="ps_r")
                    nc.tensor.matmul(
                        ps_r[:, :wdt],
                        kT[:, g, j * 128:(j + 1) * 128],
                        qT[:, h, j * 128:],
                        start=True, stop=True,
                    )
                    pt = pT_pool.tile([128, 512], BF16, name=f"pT_{h}_{j}", tag="pT")
                    nc.scalar.activation(pt[:, j * 128:], ps_r[:, :wdt],
                                         mybir.ActivationFunctionType.Exp)
                    nc.vector.tensor_tensor(
                        pt[:, j * 128:(j + 1) * 128],
                        pt[:, j * 128:(j + 1) * 128],
                        dmask[:], op=AX.mult,
                    )
                    pT_sb.append(pt)
                ps_o = psO.tile([65, 512], F32, name=f"ps_o_{h}", tag="ps_o",
                                bufs=3)
                for j in range(TT):
                    nc.tensor.matmul(
                        ps_o[:, j * 128:],
                        v_sb[:, j, g, :],
                        pT_sb[j][:, j * 128:],
                        start=(j == 0), stop=(j == TT - 1),
                    )
                lns = stat.tile([1, 512], F32, name=f"lns_{h}", tag="lns")
                nc.scalar.activation(lns[:], ps_o[64:65, :],
                                     mybir.ActivationFunctionType.Ln)
                invs = stat.tile([1, 512], mybir.dt.float32r, name=f"invs_{h}",
                                 tag="invs")
                nc.scalar.activation(invs[:], lns[:],
                                     mybir.ActivationFunctionType.Exp, scale=-1.0)
                return ps_o, invs

            def attention_head_finalize(h, ps_o, invs):
                hp = h // 2
                ho = (h % 2) * 64
                rb_ps = psO.tile([64, 512], F32, name=f"rb_ps_{h}", tag="rb_ps",
                                 bufs=1)
                nc.tensor.matmul(rb_ps[:], ones64[:], invs[:], start=True, stop=True)
                ao_raw = work.tile([64, 512], BF16, name=f"ao_raw_{h}", tag="ao_raw")
                nc.vector.tensor_copy(ao_raw[:], ps_o[0:64, :])
                nc.vector.tensor_tensor(
                    aoT[ho:ho + 64, hp, :], ao_raw[:], rb_ps[:], op=AX.mult,
                )

            pending = []
            for o in range(3):
                if o > 0:
                    wq_t = wpool.tile([128, KT, 512], BF16, name=f"wq_{o}",
                                      tag="wtile")
                    for q4 in range(4):
                        nc.sync.dma_start(
                            wq_t[:, 4 * q4:4 * q4 + 4, :],
                            wq_r[:, 4 * q4:4 * q4 + 4, o * 512:(o + 1) * 512])
                    proj_halves(wq_t, q_dest_factory(o))
                for h in range(8 * o, 8 * o + 8):
                    res = attention_head_main(h)
                    if len(pending) >= 2:
                        attention_head_finalize(*pending.pop(0))
                    pending.append((h, *res))

            # q chunk 3 projection, then wo prefetch
            wq_t3 = wpool.tile([128, KT, 512], BF16, name="wq_3", tag="wtile")
            for q4 in range(4):
                nc.sync.dma_start(wq_t3[:, 4 * q4:4 * q4 + 4, :],
                                  wq_r[:, 4 * q4:4 * q4 + 4, 3 * 512:4 * 512])
            proj_halves(wq_t3, q_dest_factory(3))

            wo_r = wo_d.rearrange("(t p) n -> p t n", p=128)
            wo_ts = []
            for o in range(4):
                w = wpool.tile([128, KT, 512], BF16, name=f"wo_{o}", tag="wtile")
                for q4 in range(4):
                    nc.sync.dma_start(
                        w[:, 4 * q4:4 * q4 + 4, :],
                        wo_r[:, 4 * q4:4 * q4 + 4, o * 512:(o + 1) * 512])
                wo_ts.append(w)

            # o_proj first half (heads 0-15 contraction) interleaved with the
            # last 8 attention heads; partials parked in DRAM
            def oproj_pass1_chunk(c):
                o, t = divmod(c, 4)
                ps = psP.tile([128, 512], F32, name=f"p1_{c}", tag=f"psq{t % 2}")
                for kt in range(8):
                    nc.tensor.matmul(
                        ps[:], aoT[:, kt, t * 128:(t + 1) * 128],
                        wo_ts[o][:, kt, :],
                        start=(kt == 0), stop=(kt == 7),
                    )
                ot = work.tile([128, 512], F32, name="out_sb", tag="out_sb")
                nc.vector.tensor_copy(ot[:], ps[:])
                nc.sync.dma_start(
                    oacc_d[t * 128:(t + 1) * 128, o * 512:(o + 1) * 512], ot[:]
                )

            c = 0
            for h in range(24, 32):
                res = attention_head_main(h)
                if len(pending) >= 2:
                    attention_head_finalize(*pending.pop(0))
                pending.append((h, *res))
                for _ in range(2):
                    oproj_pass1_chunk(c)
                    c += 1

            while pending:
                attention_head_finalize(*pending.pop(0))

            # o_proj second half + combine
            for o in range(4):
                for t in range(TT):
                    acc_in = work.tile([128, 512], F32, name="acc_in", tag="acc_in")
                    nc.sync.dma_start(
                        acc_in[:],
                        oacc_d[t * 128:(t + 1) * 128, o * 512:(o + 1) * 512],
                    )
                    ps = psP.tile([128, 512], F32, name=f"p2_{o}_{t}",
                                  tag=f"psq{t % 2}")
                    for kt in range(8, KT):
                        nc.tensor.matmul(
                            ps[:], aoT[:, kt, t * 128:(t + 1) * 128],
                            wo_ts[o][:, kt, :],
                            start=(kt == 8), stop=(kt == KT - 1),
                        )
                    ot = work.tile([128, 512], F32, name="out_sb", tag="out_sb")
                    nc.vector.tensor_tensor(ot[:], ps[:], acc_in[:], op=AX.add)
                    nc.sync.dma_start(
                        out_d[t * 128:(t + 1) * 128, o * 512:(o + 1) * 512], ot[:]
                    )

    return nc


_nc_cache = [None]


def _rope_tables():
    inv_freq = 1.0 / (ROPE_BASE ** (np.arange(0, HEAD_DIM, 2, dtype=np.float32) / HEAD_DIM))
    pos = np.arange(S, dtype=np.float32)
    freqs = np.outer(pos, inv_freq)
    emb = np.concatenate([freqs, freqs], axis=-1)  # [S, D]
    return np.cos(emb).astype(np.float32), np.sin(emb).astype(np.float32)


def prepare_in_maps(x, Wq, Wk, Wv, Wo):
    scale = np.float32(HEAD_DIM ** -0.5)
    cos, sin = _rope_tables()
    cos8 = np.ascontiguousarray(np.tile(cos, (1, N_KV_HEADS))).astype(BF)   # [S, 512]
    sin8 = np.ascontiguousarray(np.tile(sin, (1, N_KV_HEADS))).astype(BF)
    ident = np.eye(128, dtype=np.float32).astype(BF)
    dmask = np.triu(np.ones((128, 128), dtype=np.float32), k=0).astype(BF)
    wq_s = np.ascontiguousarray((Wq.astype(np.float32) * scale).astype(BF))
    wk = np.ascontiguousarray(Wk.astype(np.float32).astype(BF))
    wv = np.ascontiguousarray(Wv.astype(np.float32).astype(BF))
    wo = np.ascontiguousarray(Wo.astype(np.float32).astype(BF))
    in_maps = []
    for b in range(B):
        in_maps.append({
            "xt": np.ascontiguousarray(x[b].T.astype(np.float32)).astype(BF),
            "wq": wq_s, "wk": wk, "wv": wv, "wo": wo,
            "cos8": cos8, "sin8": sin8, "ident": ident, "dmask": dmask,
            "ones64": np.ones((1, 64), dtype=np.float32),
        })
    return in_maps


def _axon_reset():
    try:
        import ctypes
        import jax
        jax.devices()
        ctypes.CDLL("/opt/axon/libaxon_pjrt.so").axon_reset()
    except Exception:
        pass


def run(x, Wq, Wk, Wv, Wo, trace=False, **spmd_kwargs):
    if _nc_cache[0] is None:
        _nc_cache[0] = build_nc()
    nc = _nc_cache[0]
    in_maps = prepare_in_maps(x, Wq, Wk, Wv, Wo)
    try:
        res = run_bass_kernel_spmd(nc, in_maps, core_ids=list(range(B)),
                                   trace=trace, **spmd_kwargs)
    except Exception as e:
        # Rare flaky device fault (NRT_EXEC_UNIT_UNRECOVERABLE) — reset the
        # accelerator and retry once.
        if "unrecoverable" not in str(e).lower() and "UNAVAILABLE" not in str(e):
            raise
        _axon_reset()
        res = run_bass_kernel_spmd(nc, in_maps, core_ids=list(range(B)),
                                   trace=trace, **spmd_kwargs)
    out = np.stack([res.results[b]["out"] for b in range(B)])       # [B, S, H]
    kc = np.stack([res.results[b]["kc"] for b in range(B)])         # [B, Hkv, S, D]
    vc = np.stack([res.results[b]["vc"] for b in range(B)])
    return (out, kc, vc), res


def kernel(x, Wq, Wk, Wv, Wo):
    (out, kc, vc), _ = run(np.asarray(x), np.asarray(Wq), np.asarray(Wk),
                           np.asarray(Wv), np.asarray(Wo), trace=False)
    return out.astype(np.float32), kc.astype(np.float32), vc.astype(np.float32)


# revision 39
# speedup vs baseline: 1.0200x; 1.0200x over previous
"""Trainium2 Bass kernel for GQA attention (nn_Attention_75539884802796).

Sharding: data-parallel over batch — B=8 batch elements across 8 NeuronCores,
one full attention layer per core, zero collectives.

Per-core pipeline (static/unrolled, Tile-scheduled, bf16 matmul path):
  1. k/q/v projections as bf16 matmuls (xT resident, weights streamed once,
     per-k-tile DMAs so the PE starts as soon as the first tiles land)
  2. RoPE in natural [token, head*dim] layout on DVE (PSUM evacuated with one
     copy so projection banks free early); k/v caches stored in f32
  3. PE transposes q,k into per-head base-0 [64, S] layouts
  4. per head (software-pipelined, finalize lags 2 heads):
     probsT = exp(kT.T @ qT) computed chunk-wise directly in transposed
     layout (no probs transposes), causal via suffix widths + one diag
     0/1-mask multiply; AV matmul uses V with an appended ones-column so
     row 64 of the [65, 512] PSUM output accumulates softmax denominators;
     1/sums via ACT ln -> exp(-x) (DVE reciprocal is 8 cyc/elem — too slow),
     broadcast across partitions with a f32r ones-outer-product on PE,
     final normalize on DVE writes the transposed attention output
  5. o_proj with a split contraction: the first half (heads 0-15) runs
     during the last 8 attention heads with partials parked in internal
     DRAM; the second half adds them back, shrinking the serial tail
Outputs: out [S, HIDDEN] f32, k/v caches [HKV, S, D] f32 per batch.
Measured: ~282 us exec on hardware, rel err ~5e-3 (bf16 path).

Self-contained: hardcodes all shapes; no sibling imports.

Toolchain compatibility patches (this image's walrus):
  - sync waits are limited to 1 per instruction (0 for Drain/NoOp); Tile
    fuses many waits onto one instruction -> split them into standalone
    EventSemaphore instructions at BIR-JSON serialization time.
  - Tile's tail drain+barrier emits Drains carrying sync -> replaced with
    single-wait instructions and sem-only barriers.
  - gpsimd cast-DMAs and custom gpsimd ISA ops (partition_broadcast) crash
    or fail codegen here — avoided by design.
"""

import ml_dtypes
import numpy as np
import orjson

import concourse.bass as bass
import concourse.mybir as mybir
from concourse.bass_utils import run_bass_kernel_spmd
from concourse.tile import TileContext
from concourse.vector_clock import ScopedClock

HIDDEN = 2048
N_HEADS = 32
N_KV_HEADS = 8
HEAD_DIM = 64
S = 512
B = 8
ROPE_BASE = 10000.0

KT = HIDDEN // 128        # 16 contraction tiles
TT = S // 128             # 4 token tiles
QCH = N_HEADS * HEAD_DIM // 512   # 4 q-projection column chunks (8 heads each)
GROUPS = N_HEADS // N_KV_HEADS

F32 = mybir.dt.float32
BF16 = mybir.dt.bfloat16
AX = mybir.AluOpType
BF = ml_dtypes.bfloat16


# --------------------------------------------------------------------------
# toolchain compatibility patches
# --------------------------------------------------------------------------

def _patch_tile_tail():
    if getattr(TileContext, "_tail_patched", False):
        return

    def patched(self, tick_clock, wait_clock):
        nc = self.nc
        tmp = nc.sync.nop(nofuse=True)
        wait_clock.add_sem_waits(tmp.ins, ScopedClock({None: tick_clock.global_clock}))
        waits = list(tmp.ins.sync_info.on_wait)
        del tmp.ins.sync_info.on_wait[:]
        id2sem = {sem.num: sem for sem in self.sems.allocated().values()}
        for w in waits:
            sem = id2sem.get(w.id)
            assert sem is not None, f"unknown sem id {w.id}"
            nc.sync.wait_ge(sem, w.wait_value)
        nc.all_engine_barrier(sem_only=True)
        popped = nc._tile_sem_poison_stack.pop()
        assert popped is self._sem_poison
        nc.clear_and_free_semaphores(list(self.sems.allocated().values()))
        nc.all_engine_barrier(sem_only=True)

    TileContext._drain_and_barrier = patched
    TileContext._tail_patched = True


def _split_multi_waits(bir: dict) -> dict:
    """Walrus here accepts at most one sync wait per instruction (none on
    Drain/NoOp). Hoist extra waits onto standalone EventSemaphore
    instructions inserted just before, on the same engine."""
    n_new = 0
    for fn in bir.get("functions", []):
        for blk in fn.get("blocks", []):
            insts = blk.get("instructions")
            if not insts:
                continue
            out = []
            for inst in insts:
                si = inst.get("sync_info")
                waits = (si or {}).get("on_wait") or []
                keep = 0 if inst.get("opcode") in ("Drain", "NoOp") else 1
                if len(waits) > keep:
                    split = waits[: len(waits) - keep]
                    si["on_wait"] = waits[len(waits) - keep:]
                    for w in split:
                        n_new += 1
                        out.append({
                            "debug": inst.get("debug", {}),
                            "engine": inst["engine"],
                            "ins": [],
                            "name": f"{inst['name']}_sw{n_new}",
                            "opcode": "EventSemaphore",
                            "outs": [],
                            "sync_info": {"on_update": [], "on_wait": [w]},
                        })
                out.append(inst)
            blk["instructions"] = out
    return bir


def _patch_to_json():
    if getattr(bass.Bass, "_json_multiwait_patched", False):
        return
    orig = bass.Bass.to_json_bytes

    def patched(self):
        data = orig(self)
        bir = orjson.loads(data)
        bir = _split_multi_waits(bir)
        return orjson.dumps(bir)

    bass.Bass.to_json_bytes = patched
    bass.Bass._json_multiwait_patched = True


def apply_patches():
    _patch_tile_tail()
    _patch_to_json()


# --------------------------------------------------------------------------
# kernel graph
# --------------------------------------------------------------------------

def build_nc():
    """Build the per-core Bass graph (same graph on all 8 cores)."""
    apply_patches()
    nc = bass.Bass("TRN2", target_bir_lowering=False)

    xt_d = nc.dram_tensor("xt", [HIDDEN, S], BF16, kind="ExternalInput")
    wq_d = nc.dram_tensor("wq", [HIDDEN, N_HEADS * HEAD_DIM], BF16, kind="ExternalInput")
    wk_d = nc.dram_tensor("wk", [HIDDEN, N_KV_HEADS * HEAD_DIM], BF16, kind="ExternalInput")
    wv_d = nc.dram_tensor("wv", [HIDDEN, N_KV_HEADS * HEAD_DIM], BF16, kind="ExternalInput")
    wo_d = nc.dram_tensor("wo", [N_HEADS * HEAD_DIM, HIDDEN], BF16, kind="ExternalInput")
    cos_d = nc.dram_tensor("cos8", [S, 512], BF16, kind="ExternalInput")
    sin_d = nc.dram_tensor("sin8", [S, 512], BF16, kind="ExternalInput")
    id_d = nc.dram_tensor("ident", [128, 128], BF16, kind="ExternalInput")
    dm_d = nc.dram_tensor("dmask", [128, 128], BF16, kind="ExternalInput")
    on_d = nc.dram_tensor("ones64", [1, 64], mybir.dt.float32r, kind="ExternalInput")

    out_d = nc.dram_tensor("out", [S, HIDDEN], F32, kind="ExternalOutput")
    oacc_d = nc.dram_tensor("oacc", [S, HIDDEN], F32)
    kc_d = nc.dram_tensor("kc", [N_KV_HEADS, S, HEAD_DIM], F32, kind="ExternalOutput")
    vc_d = nc.dram_tensor("vc", [N_KV_HEADS, S, HEAD_DIM], F32, kind="ExternalOutput")

    F32R = mybir.dt.float32r

    with TileContext(nc) as tc:
        with (
            tc.tile_pool(name="const", bufs=1) as cpool,
            tc.tile_pool(name="resident", bufs=1) as rpool,
            tc.tile_pool(name="wstream", bufs=4) as wpool,
            tc.tile_pool(name="work", bufs=3) as work,
            tc.tile_pool(name="probsT_sb", bufs=8) as pT_pool,
            tc.tile_pool(name="stats", bufs=2) as stat,
            tc.tile_pool(name="psProj", bufs=1, space="PSUM") as psP,
            tc.tile_pool(name="psR", bufs=2, space="PSUM") as psR,
            tc.tile_pool(name="psO", bufs=2, space="PSUM") as psO,
        ):
            # ---- constants / resident tensors ----
            ident = cpool.tile([128, 128], BF16, name="ident")
            nc.sync.dma_start(ident[:], id_d[:])
            dmask = cpool.tile([128, 128], BF16, name="dmask")
            nc.sync.dma_start(dmask[:], dm_d[:])
            ones64 = cpool.tile([1, 64], F32R, name="ones64")
            nc.sync.dma_start(ones64[:], on_d[:])
            xt = rpool.tile([128, KT, S], BF16, name="xt")
            xt_r = xt_d.rearrange("(t p) m -> p t m", p=128)
            wq_r = wq_d.rearrange("(t p) n -> p t n", p=128)
            wk_t0 = wpool.tile([128, KT, 512], BF16, name="wk", tag="wtile")
            wk_r0 = wk_d.rearrange("(t p) n -> p t n", p=128)
            wq_t00 = wpool.tile([128, KT, 512], BF16, name="wq_0", tag="wtile")
            for kt in range(KT):
                nc.sync.dma_start(xt[:, kt, :], xt_r[:, kt, :])
                nc.sync.dma_start(wk_t0[:, kt, :], wk_r0[:, kt, :])
            for q4 in range(4):
                nc.sync.dma_start(wq_t00[:, 4 * q4:4 * q4 + 4, :],
                                  wq_r[:, 4 * q4:4 * q4 + 4, 0:512])
            cos8 = cpool.tile([128, TT, 512], BF16, name="cos8")
            nc.sync.dma_start(cos8[:], cos_d.rearrange("(t p) n -> p t n", p=128))
            sin8 = cpool.tile([128, TT, 512], BF16, name="sin8")
            nc.sync.dma_start(sin8[:], sin_d.rearrange("(t p) n -> p t n", p=128))
            wv_t0 = wpool.tile([128, KT, 512], BF16, name="wv", tag="wtile")
            wv_r0 = wv_d.rearrange("(t p) n -> p t n", p=128)
            for q4 in range(4):
                nc.sync.dma_start(wv_t0[:, 4 * q4:4 * q4 + 4, :],
                                  wv_r0[:, 4 * q4:4 * q4 + 4, :])

            qT = rpool.tile([128, N_HEADS // 2, S], BF16, name="qT")
            kT = rpool.tile([128, N_KV_HEADS, S], BF16, name="kT")
            v_sb = rpool.tile([128, TT, N_KV_HEADS, 65], BF16, name="v_sb")
            aoT = rpool.tile([128, KT, S], BF16, name="aoT")
            nc.vector.memset(v_sb[:, :, :, 64:65], 1.0)

            def rope_block(ps_in, cos_t, sin_t, out_tile):
                raw = work.tile([128, 512], F32, name="roperaw", tag="roperaw")
                nc.vector.tensor_copy(raw[:], ps_in)
                qc = work.tile([128, 512], F32, name="ropeqc", tag="ropeqc")
                nc.vector.tensor_tensor(qc[:], raw[:], cos_t, op=AX.mult)
                ps3 = raw.rearrange("p (g c) -> p g c", c=64)
                qc3 = qc.rearrange("p (g c) -> p g c", c=64)
                sn3 = sin_t.rearrange("p (g c) -> p g c", c=64)
                ot3 = out_tile.rearrange("p (g c) -> p g c", c=64)
                t1 = work.tile([128, 256], F32, name="ropet1", tag="ropet1")
                t13 = t1.rearrange("p (g c) -> p g c", c=32)
                nc.vector.tensor_tensor(t13[:], ps3[:, :, 32:64], sn3[:, :, 0:32], op=AX.mult)
                nc.vector.tensor_tensor(ot3[:, :, 0:32], qc3[:, :, 0:32], t13[:], op=AX.subtract)
                t2 = work.tile([128, 256], F32, name="ropet2", tag="ropet2")
                t23 = t2.rearrange("p (g c) -> p g c", c=32)
                nc.vector.tensor_tensor(t23[:], ps3[:, :, 0:32], sn3[:, :, 32:64], op=AX.mult)
                nc.vector.tensor_tensor(ot3[:, :, 32:64], qc3[:, :, 32:64], t23[:], op=AX.add)

            def proj_halves(w_tile, dest_cb):
                """Run a [*,512]-wide projection in two tt-halves on 2 PSUM
                banks; dest_cb(t, ps) consumes each token-tile's PSUM."""
                for half in range(2):
                    ps2 = [psP.tile([128, 512], F32, name=f"pp_{nc.next_id()}",
                                    tag=f"psq{t}") for t in range(2)]
                    for kt in range(KT):
                        for ti in range(2):
                            t = 2 * half + ti
                            nc.tensor.matmul(
                                ps2[ti][:], xt[:, kt, t * 128:(t + 1) * 128],
                                w_tile[:, kt, :],
                                start=(kt == 0), stop=(kt == KT - 1),
                            )
                    for ti in range(2):
                        dest_cb(2 * half + ti, ps2[ti])

            # ================= k projection =================
            wk_t = wk_t0
            kc_r = kc_d.rearrange("g s d -> s g d")

            def k_dest(t, ps_k):
                krot_f = work.tile([128, 512], F32, name="krot_f", tag="rot_f")
                rope_block(ps_k[:], cos8[:, t, :], sin8[:, t, :], krot_f)
                nc.sync.dma_start(
                    kc_r[t * 128:(t + 1) * 128, :, :],
                    krot_f.rearrange("p (g d) -> p g d", d=64),
                )
                krot = work.tile([128, 512], BF16, name="krot", tag="rot")
                nc.vector.tensor_copy(krot[:], krot_f[:])
                ps_trk = psO.tile([64, 8, 128], BF16, name=f"ps_trk_{t}",
                                  tag="rb_ps", bufs=1)
                for gg in range(8):
                    nc.tensor.transpose(ps_trk[:, gg, :],
                                        krot[:, gg * 64:(gg + 1) * 64], ident[:])
                nc.vector.tensor_copy(kT[0:64, :, t * 128:(t + 1) * 128], ps_trk[:])
                nc.vector.tensor_copy(kT[64:128, :, t * 128:(t + 1) * 128], ps_trk[:])

            proj_halves(wk_t, k_dest)

            # ================= q chunk 0 projection =================
            def q_dest_factory(o):
                def q_dest(t, ps_q):
                    qrot = work.tile([128, 512], BF16, name="qrot", tag="rot")
                    rope_block(ps_q[:], cos8[:, t, :], sin8[:, t, :], qrot)
                    ps_tr = psO.tile([64, 8, 128], BF16,
                                      name=f"ps_trq_{o}_{t}", tag="rb_ps", bufs=1)
                    for hh in range(8):
                        nc.tensor.transpose(ps_tr[:, hh, :],
                                            qrot[:, hh * 64:(hh + 1) * 64], ident[:])
                    tr4 = ps_tr.rearrange("p (a two) c -> p a two c", two=2)
                    nc.vector.tensor_copy(
                        qT[0:64, 4 * o:4 * o + 4, t * 128:(t + 1) * 128],
                        tr4[:, :, 0, :],
                    )
                    nc.vector.tensor_copy(
                        qT[64:128, 4 * o:4 * o + 4, t * 128:(t + 1) * 128],
                        tr4[:, :, 1, :],
                    )
                return q_dest

            proj_halves(wq_t00, q_dest_factory(0))

            # ================= v projection =================
            wv_t = wv_t0
            vc_r = vc_d.rearrange("g s d -> s g d")

            def v_dest(t, ps_v):
                v_f = work.tile([128, 512], F32, name="v_f", tag="rot_f")
                nc.vector.tensor_copy(v_f[:], ps_v[:])
                nc.sync.dma_start(
                    vc_r[t * 128:(t + 1) * 128, :, :],
                    v_f.rearrange("p (g d) -> p g d", d=64),
                )
                nc.vector.tensor_copy(
                    v_sb[:, t, :, 0:64], v_f.rearrange("p (g d) -> p g d", d=64),
                )

            proj_halves(wv_t, v_dest)

            # ======== remaining q chunks interleaved with attention ========
            def attention_pair_chunks(p):
                """scoresT+exp for heads (2p, 2p+1) with chunk-adjacent matmuls
                in disjoint PE row groups (even head rows 0-63, odd 64-127)."""
                g = (2 * p) // GROUPS
                pT2 = [[], []]
                for j in range(TT):
                    wdt = 512 - j * 128
                    for u in range(2):
                        ho = u * 64
                        ps_r = psR.tile([128, 512], F32, name=f"ps_r_{p}_{u}_{j}",
                                        tag="ps_r")
                        nc.tensor.matmul(
                            ps_r[:, :wdt],
                            kT[ho:ho + 64, g, j * 128:(j + 1) * 128],
                            qT[ho:ho + 64, p, j * 128:],
                            start=True, stop=True,
                        )
                        pt = pT_pool.tile([128, 512], BF16, name=f"pT_{p}_{u}_{j}",
                                          tag="pT")
                        nc.scalar.activation(pt[:, j * 128:], ps_r[:, :wdt],
                                             mybir.ActivationFunctionType.Exp)
                        nc.vector.tensor_tensor(
                            pt[:, j * 128:(j + 1) * 128],
                            pt[:, j * 128:(j + 1) * 128],
                            dmask[:], op=AX.mult,
                        )
                        pT2[u].append(pt)
                return pT2

            def attention_head_av(h, pT_sb):
                g = h // GROUPS
                ps_o = psO.tile([65, 512], F32, name=f"ps_o_{h}", tag="ps_o",
                                bufs=3)
                for j in range(TT):
                    nc.tensor.matmul(
                        ps_o[:, j * 128:],
                        v_sb[:, j, g, :],
                        pT_sb[j][:, j * 128:],
                        start=(j == 0), stop=(j == TT - 1),
                    )
                lns = stat.tile([1, 512], F32, name=f"lns_{h}", tag="lns")
                nc.scalar.activation(lns[:], ps_o[64:65, :],
                                     mybir.ActivationFunctionType.Ln)
                invs = stat.tile([1, 512], mybir.dt.float32r, name=f"invs_{h}",
                                 tag="invs")
                nc.scalar.activation(invs[:], lns[:],
                                     mybir.ActivationFunctionType.Exp, scale=-1.0)
                return ps_o, invs

            def attention_head_finalize(h, ps_o, invs):
                hp = h // 2
                ho = (h % 2) * 64
                rb_ps = psO.tile([64, 512], F32, name=f"rb_ps_{h}", tag="rb_ps",
                                 bufs=1)
                nc.tensor.matmul(rb_ps[:], ones64[:], invs[:], start=True, stop=True)
                ao_raw = work.tile([64, 512], BF16, name=f"ao_raw_{h}", tag="ao_raw")
                nc.vector.tensor_copy(ao_raw[:], ps_o[0:64, :])
                nc.vector.tensor_tensor(
                    aoT[ho:ho + 64, hp, :], ao_raw[:], rb_ps[:], op=AX.mult,
                )

            pending = []
            for o in range(3):
                if o > 0:
                    wq_t = wpool.tile([128, KT, 512], BF16, name=f"wq_{o}",
                                      tag="wtile")
                    for q4 in range(4):
                        nc.sync.dma_start(
                            wq_t[:, 4 * q4:4 * q4 + 4, :],
                            wq_r[:, 4 * q4:4 * q4 + 4, o * 512:(o + 1) * 512])
                    proj_halves(wq_t, q_dest_factory(o))
                for p in range(4 * o, 4 * o + 4):
                    if pending:
                        attention_head_finalize(*pending.pop(0))
                    pT2 = attention_pair_chunks(p)
                    if pending:
                        attention_head_finalize(*pending.pop(0))
                    for u in range(2):
                        h = 2 * p + u
                        res = attention_head_av(h, pT2[u])
                        pending.append((h, *res))

            # q chunk 3 projection, then wo prefetch
            wq_t3 = wpool.tile([128, KT, 512], BF16, name="wq_3", tag="wtile")
            for q4 in range(4):
                nc.sync.dma_start(wq_t3[:, 4 * q4:4 * q4 + 4, :],
                                  wq_r[:, 4 * q4:4 * q4 + 4, 3 * 512:4 * 512])
            proj_halves(wq_t3, q_dest_factory(3))

            wo_r = wo_d.rearrange("(t p) n -> p t n", p=128)
            wo_ts = []
            for o in range(4):
                w = wpool.tile([128, KT, 512], BF16, name=f"wo_{o}", tag="wtile")
                for q4 in range(4):
                    nc.sync.dma_start(
                        w[:, 4 * q4:4 * q4 + 4, :],
                        wo_r[:, 4 * q4:4 * q4 + 4, o * 512:(o + 1) * 512])
                wo_ts.append(w)

            # o_proj first half (heads 0-15 contraction) interleaved with the
            # last 8 attention heads; partials parked in DRAM
            def oproj_pass1_chunk(c):
                o, t = divmod(c, 4)
                ps = psP.tile([128, 512], F32, name=f"p1_{c}", tag=f"psq{t % 2}")
                for kt in range(8):
                    nc.tensor.matmul(
                        ps[:], aoT[:, kt, t * 128:(t + 1) * 128],
                        wo_ts[o][:, kt, :],
                        start=(kt == 0), stop=(kt == 7),
                    )
                ot = work.tile([128, 512], F32, name="out_sb", tag="out_sb")
                nc.vector.tensor_copy(ot[:], ps[:])
                nc.sync.dma_start(
                    oacc_d[t * 128:(t + 1) * 128, o * 512:(o + 1) * 512], ot[:]
                )

            c = 0
            for p in range(12, 16):
                if pending:
                    attention_head_finalize(*pending.pop(0))
                pT2 = attention_pair_chunks(p)
                if pending:
                    attention_head_finalize(*pending.pop(0))
                for u in range(2):
                    h = 2 * p + u
                    res = attention_head_av(h, pT2[u])
                    pending.append((h, *res))
                    for _ in range(2):
                        oproj_pass1_chunk(c)
                        c += 1

            while pending:
                attention_head_finalize(*pending.pop(0))

            # o_proj second half + combine
            for o in range(4):
                for t in range(TT):
                    acc_in = work.tile([128, 512], F32, name="acc_in", tag="acc_in")
                    nc.sync.dma_start(
                        acc_in[:],
                        oacc_d[t * 128:(t + 1) * 128, o * 512:(o + 1) * 512],
                    )
                    ps = psP.tile([128, 512], F32, name=f"p2_{o}_{t}",
                                  tag=f"psq{t % 2}")
                    for kt in range(8, KT):
                        nc.tensor.matmul(
                            ps[:], aoT[:, kt, t * 128:(t + 1) * 128],
                            wo_ts[o][:, kt, :],
                            start=(kt == 8), stop=(kt == KT - 1),
                        )
                    ot = work.tile([128, 512], F32, name="out_sb", tag="out_sb")
                    nc.vector.tensor_tensor(ot[:], ps[:], acc_in[:], op=AX.add)
                    nc.sync.dma_start(
                        out_d[t * 128:(t + 1) * 128, o * 512:(o + 1) * 512], ot[:]
                    )

    return nc


_nc_cache = [None]


def _rope_tables():
    inv_freq = 1.0 / (ROPE_BASE ** (np.arange(0, HEAD_DIM, 2, dtype=np.float32) / HEAD_DIM))
    pos = np.arange(S, dtype=np.float32)
    freqs = np.outer(pos, inv_freq)
    emb = np.concatenate([freqs, freqs], axis=-1)  # [S, D]
    return np.cos(emb).astype(np.float32), np.sin(emb).astype(np.float32)


def prepare_in_maps(x, Wq, Wk, Wv, Wo):
    scale = np.float32(HEAD_DIM ** -0.5)
    cos, sin = _rope_tables()
    cos8 = np.ascontiguousarray(np.tile(cos, (1, N_KV_HEADS))).astype(BF)   # [S, 512]
    sin8 = np.ascontiguousarray(np.tile(sin, (1, N_KV_HEADS))).astype(BF)
    ident = np.eye(128, dtype=np.float32).astype(BF)
    dmask = np.triu(np.ones((128, 128), dtype=np.float32), k=0).astype(BF)
    wq_s = np.ascontiguousarray((Wq.astype(np.float32) * scale).astype(BF))
    wk = np.ascontiguousarray(Wk.astype(np.float32).astype(BF))
    wv = np.ascontiguousarray(Wv.astype(np.float32).astype(BF))
    wo = np.ascontiguousarray(Wo.astype(np.float32).astype(BF))
    in_maps = []
    for b in range(B):
        in_maps.append({
            "xt": np.ascontiguousarray(x[b].T.astype(np.float32)).astype(BF),
            "wq": wq_s, "wk": wk, "wv": wv, "wo": wo,
            "cos8": cos8, "sin8": sin8, "ident": ident, "dmask": dmask,
            "ones64": np.ones((1, 64), dtype=np.float32),
        })
    return in_maps


def _axon_reset():
    try:
        import ctypes
        import jax
        jax.devices()
        ctypes.CDLL("/opt/axon/libaxon_pjrt.so").axon_reset()
    except Exception:
        pass


def run(x, Wq, Wk, Wv, Wo, trace=False, **spmd_kwargs):
    if _nc_cache[0] is None:
        _nc_cache[0] = build_nc()
    nc = _nc_cache[0]
    in_maps = prepare_in_maps(x, Wq, Wk, Wv, Wo)
    try:
        res = run_bass_kernel_spmd(nc, in_maps, core_ids=list(range(B)),
                                   trace=trace, **spmd_kwargs)
    except Exception as e:
        # Rare flaky device fault (NRT_EXEC_UNIT_UNRECOVERABLE) — reset the
        # accelerator and retry once.
        if "unrecoverable" not in str(e).lower() and "UNAVAILABLE" not in str(e):
            raise
        _axon_reset()
        res = run_bass_kernel_spmd(nc, in_maps, core_ids=list(range(B)),
                                   trace=trace, **spmd_kwargs)
    out = np.stack([res.results[b]["out"] for b in range(B)])       # [B, S, H]
    kc = np.stack([res.results[b]["kc"] for b in range(B)])         # [B, Hkv, S, D]
    vc = np.stack([res.results[b]["vc"] for b in range(B)])
    return (out, kc, vc), res


def kernel(x, Wq, Wk, Wv, Wo):
    (out, kc, vc), _ = run(np.asarray(x), np.asarray(Wq), np.asarray(Wk),
                           np.asarray(Wv), np.asarray(Wo), trace=False)
    return out.astype(np.float32), kc.astype(np.float32), vc.astype(np.float32)
